# revision 12
# baseline (speedup 1.0000x reference)
"""Trainium2 Bass kernel for the BiLSTM-CRF loss (sum reduction).

Strategy (v2):
- Data-parallel: batch 256 sharded as 32 per NeuronCore across 8 cores.
- Normalizer (forward algorithm) in LINEAR space: alpha_{s+1} =
  exp(em_{s+1}) .* (E^T alpha_s), E = exp(transitions). 32 concurrent
  segments of 16 steps (Birkhoff contraction => 6 burn-in rounds on the
  previous segment's tail) -> 22 matmul rounds of width 1024.
- Emission pipeline: slot-major loads (slot j = every 16th timestep) as
  [128,8,128] f32 tiles with 4KB contiguous HBM lines -> scalar Exp
  f32->bf16 in natural layout -> DMA-xbar transpose (bf16) directly into
  erm[t, seg*32+b]. No PE transposes, no DVE casts; the serial chain is
  gated only by the HBM stream.
- Per-segment growth via boundary column sums; fp32 range kept by 2
  delayed column rescales.
- Numerator: two indirect-DMA element gathers + reductions, overlapped.

kernel() contract: full unsharded inputs in, full output (scalar) out.
"""
import numpy as np

S, B, T = 512, 256, 128
NCORES, Bl = 8, 32
NSEG, SEGLEN = 32, 16
BURN = 6
NR = BURN + SEGLEN                   # 22 rounds
NSLOT = SEGLEN
RESC_APPLY = [10, 16]
C_RESC = 2.0 ** -46
RESC_LOGSUM = len(RESC_APPLY) * 46 * float(np.log(2.0))
INIT_BURN = 2.0 ** -30
TSSE_N = T * T + T + T + 1           # 16641: trans | start | end | 0.0
TSSE_PAD = TSSE_N - 1
# slots in need-time order: burn-in tails first, then slot 0, then 1..10
LOAD_ORDER = [11, 12, 13, 14, 15] + list(range(11))

_NC = None


def _build():
    import concourse.bass as bass
    import concourse.tile as tile
    from concourse import bacc, mybir
    from contextlib import ExitStack

    f32 = mybir.dt.float32
    bf16 = mybir.dt.bfloat16
    i32 = mybir.dt.int32
    AF = mybir.ActivationFunctionType
    OP = mybir.AluOpType
    AX = mybir.AxisListType

    nc = bacc.Bacc("TRN2", target_bir_lowering=False, debug=False,
                   num_devices=NCORES)

    em = nc.dram_tensor("em", [S, Bl, T], f32, kind="ExternalInput")
    transm = nc.dram_tensor("transm", [T, T], f32, kind="ExternalInput")
    startv = nc.dram_tensor("startv", [T, 1], f32, kind="ExternalInput")
    endv = nc.dram_tensor("endv", [T, 1], f32, kind="ExternalInput")
    emidx = nc.dram_tensor("emidx", [128, 128], i32, kind="ExternalInput")
    tssev = nc.dram_tensor("tssev", [TSSE_N, 1], f32, kind="ExternalInput")
    tsseidx = nc.dram_tensor("tsseidx", [128, 129], i32, kind="ExternalInput")
    outv = nc.dram_tensor("out", [1, 1], f32, kind="ExternalOutput")

    H = NSEG // 2

    with tile.TileContext(nc) as tc, ExitStack() as ctx:
        const = ctx.enter_context(tc.tile_pool(name="const", bufs=1))
        stage = ctx.enter_context(tc.tile_pool(name="stage", bufs=4))
        pchain = ctx.enter_context(tc.tile_pool(name="pchain", bufs=2,
                                                space="PSUM"))
        pstat = ctx.enter_context(tc.tile_pool(name="pstat", bufs=2,
                                               space="PSUM"))

        # ---------- constants ----------
        ones_col = const.tile([128, 1], bf16)
        nc.vector.memset(ones_col[:], 1.0)
        ones_colf = const.tile([128, 1], f32)
        nc.vector.memset(ones_colf[:], 1.0)

        tr_sb = const.tile([128, 128], f32)
        nc.sync.dma_start(out=tr_sb[:], in_=transm[:, :])
        E_hi = const.tile([128, 128], bf16)
        nc.scalar.activation(E_hi[:], tr_sb[:], AF.Exp)
        st_sb = const.tile([128, 1], f32)
        nc.sync.dma_start(out=st_sb[:], in_=startv[:, :])
        Estart = const.tile([128, 1], f32)
        nc.scalar.activation(Estart[:], st_sb[:], AF.Exp)
        en_sb = const.tile([128, 1], f32)
        nc.sync.dma_start(out=en_sb[:], in_=endv[:, :])
        Eend = const.tile([128, 1], bf16)
        nc.scalar.activation(Eend[:], en_sb[:], AF.Exp)

        # ---------- numerator: indirect gathers ----------
        emidx_sb = const.tile([128, 128], i32)
        nc.sync.dma_start(out=emidx_sb[:], in_=emidx[:, :])
        tsseidx_sb = const.tile([128, 129], i32)
        nc.sync.dma_start(out=tsseidx_sb[:], in_=tsseidx[:, :])
        gem = const.tile([128, 128], f32)
        nc.gpsimd.indirect_dma_start(
            out=gem[:], out_offset=None,
            in_=bass.AP(tensor=em, offset=0,
                        ap=[[1, S * Bl * T], [1, 1]]),
            in_offset=bass.IndirectOffsetOnAxis(ap=emidx_sb[:], axis=0))
        gts = const.tile([128, 129], f32)
        nc.gpsimd.indirect_dma_start(
            out=gts[:], out_offset=None,
            in_=bass.AP(tensor=tssev, offset=0,
                        ap=[[1, TSSE_N], [1, 1]]),
            in_offset=bass.IndirectOffsetOnAxis(ap=tsseidx_sb[:], axis=0))

        # ---------- chain state + emission storage ----------
        # Band-major free layout: chain column u = 128*bl + 4*g + bh where
        # the batch index b = 8*bh + bl and g is the segment. Segment g
        # occupies p = u%128 in [4g, 4g+4) for each band bl = u//128.
        A2 = const.tile([128, NSEG * Bl], bf16)
        nc.vector.memset(A2[:], INIT_BURN)
        A2v = A2.rearrange("t (g p) -> t g p", g=8)
        erm = const.tile([128, NSLOT, NSEG * Bl], bf16)
        ermv = erm.rearrange("t s (g p) -> t s g p", g=8)

        n_sb = const.tile([1, NSEG * Bl], f32)
        m_sb = const.tile([1, NSEG * Bl], f32)
        fin_sb = const.tile([1, Bl], f32)
        n_sbv = n_sb.rearrange("q (g p) -> q g p", g=8)
        m_sbv = m_sb.rearrange("q (g p) -> q g p", g=8)

        # em[s=16g+j, b=8*bh+bl, t] -> slot j, DRAM iter (g, bh, bl*t);
        # SBUF row r = g*32+b lands at partition r//8, band r%8.
        emr = em[:, :, :].rearrange("(g j) (bh bl) t -> j g bh (bl t)",
                                    j=NSLOT, bh=4)

        def emit_load(j, eng):
            nat = stage.tile([128, 8, 128], f32, tag="nat")
            eng.dma_start(out=nat[:], in_=emr[j])
            return nat

        def emit_expt(j, nat):
            natb = stage.tile([128, 8, 128], bf16, tag="natb")
            nc.scalar.activation(natb[:], nat[:], AF.Exp)
            # xbar transpose band gi: erm[t, j, 128*gi + p] = natb[p, gi, t]
            for gi in range(8):
                nc.sync.dma_start_transpose(
                    out=erm[:, j, 128 * gi:128 * (gi + 1)],
                    in_=natb[:, gi, :])

        def emit_round(r):
            if r < BURN - 1:
                slot, sh = SEGLEN - (BURN - 1) + r, -4
            elif r == BURN - 1:
                slot, sh = 0, 0
            else:
                jj = r - BURN
                slot = (jj + 1) % SEGLEN
                sh = 4 if jj == SEGLEN - 1 else 0
            if r < BURN:
                psl = [(4, 64), (64, 128)]
            elif r < NR - 1:
                psl = [(0, 64), (64, 128)]
            else:
                psl = [(0, 64), (64, 124)]
            for (pa, pb), tg in zip(psl, ("psA", "psB")):
                pw = pb - pa
                ps = pchain.tile([128, 8 * 64], f32, tag=tg)
                nc.tensor.matmul(out=ps[:, :8 * pw], lhsT=E_hi[:],
                                 rhs=A2v[:, :, pa:pb],
                                 start=True, stop=True)
                nc.vector.tensor_tensor(
                    out=A2v[:, :, pa:pb],
                    in0=ps[:, :8 * pw].rearrange("t (g p) -> t g p", p=pw),
                    in1=ermv[:, slot, :, pa + sh:pb + sh],
                    op=OP.mult)
            if r in RESC_APPLY:
                nc.vector.tensor_scalar_mul(A2[:], A2[:], C_RESC)
            if r == BURN - 1:
                for h, tg in enumerate(("st0", "st1")):
                    cs = pstat.tile([1, 8 * 64], f32, tag=tg)
                    nc.tensor.matmul(out=cs[:], lhsT=ones_col[:],
                                     rhs=A2v[:, :, 64 * h:64 * (h + 1)],
                                     start=True, stop=True)
                    nc.vector.tensor_copy(
                        out=n_sbv[:, :, 64 * h:64 * (h + 1)],
                        in_=cs.rearrange("q (g p) -> q g p", p=64))
            if r == NR - 2:
                m31 = pstat.tile([1, 8 * 64], f32, tag="st0")
                nc.tensor.matmul(out=m31[:, :32], lhsT=ones_col[:],
                                 rhs=A2v[:, :, 124:128],
                                 start=True, stop=True)
                nc.vector.tensor_copy(
                    out=m_sbv[:, :, 124:128],
                    in_=m31[:, :32].rearrange("q (g p) -> q g p", p=4))
                fin = pstat.tile([1, 8 * 64], f32, tag="st1")
                nc.tensor.matmul(out=fin[:, :32], lhsT=Eend[:],
                                 rhs=A2v[:, :, 124:128],
                                 start=True, stop=True)
                nc.vector.tensor_copy(out=fin_sb[:], in_=fin[:, :32])
            if r == NR - 1:
                mm0 = pstat.tile([1, 8 * 64], f32, tag="st0")
                nc.tensor.matmul(out=mm0[:], lhsT=ones_col[:],
                                 rhs=A2v[:, :, 0:64], start=True, stop=True)
                nc.vector.tensor_copy(
                    out=m_sbv[:, :, 0:64],
                    in_=mm0.rearrange("q (g p) -> q g p", p=64))
                mm1 = pstat.tile([1, 8 * 64], f32, tag="st1")
                nc.tensor.matmul(out=mm1[:, :8 * 60], lhsT=ones_col[:],
                                 rhs=A2v[:, :, 64:124],
                                 start=True, stop=True)
                nc.vector.tensor_copy(
                    out=m_sbv[:, :, 64:124],
                    in_=mm1[:, :8 * 60].rearrange("q (g p) -> q g p", p=60))

        # ---------- emission + chain ----------
        engs = [nc.scalar, nc.gpsimd]
        LA = 3  # load lookahead; must stay < stage bufs to avoid deadlock
        nats = {}
        for i in range(len(LOAD_ORDER) + LA):
            if i < len(LOAD_ORDER):
                j = LOAD_ORDER[i]
                nats[j] = emit_load(j, engs[i % 2])
            if i >= LA:
                jj = LOAD_ORDER[i - LA]
                emit_expt(jj, nats[jj])
        for r in range(BURN - 1):
            emit_round(r)
        # alpha_0 = start .* exp(em_0) replaces seg 0's burn-in state
        nc.vector.tensor_scalar_mul(A2v[:, :, 0:4], ermv[:, 0, :, 0:4],
                                    Estart[:])
        for r in range(BURN - 1, NR):
            emit_round(r)

        # ---------- final assembly ----------
        gsum1 = const.tile([128, 1], f32)
        nc.vector.reduce_sum(out=gsum1[:], in_=gem[:], axis=AX.X)
        gsum2 = const.tile([128, 1], f32)
        nc.vector.reduce_sum(out=gsum2[:], in_=gts[:], axis=AX.X)
        numcol = const.tile([128, 1], f32)
        nc.vector.tensor_add(out=numcol[:], in0=gsum1[:], in1=gsum2[:])
        logn = const.tile([1, NSEG * Bl], f32)
        nc.scalar.activation(logn[:], n_sb[:], AF.Ln)
        logm = const.tile([1, NSEG * Bl], f32)
        nc.scalar.activation(logm[:], m_sb[:], AF.Ln)
        grow = const.tile([1, NSEG * Bl], f32)
        nc.vector.tensor_tensor(out=grow[:], in0=logm[:], in1=logn[:],
                                op=OP.subtract)
        nc.vector.tensor_scalar_add(grow[:], grow[:], RESC_LOGSUM)
        # band-major u = 128*bl + 4*g + bh; reduce over g, out b' = 4*bl+bh
        growb = const.tile([1, Bl], f32)
        nc.vector.reduce_sum(
            out=growb[:],
            in_=grow.rearrange("q (bl g bh) -> q bl bh g", bl=8, bh=4),
            axis=AX.X)
        logfin = const.tile([1, Bl], f32)
        nc.scalar.activation(logfin[:], fin_sb[:], AF.Ln)
        lz = const.tile([1, Bl], f32)
        nc.vector.tensor_add(out=lz[:], in0=growb[:], in1=logfin[:])
        logmv = logm.rearrange("q (g p) -> q g p", g=8)
        lognv = logn.rearrange("q (g p) -> q g p", g=8)
        nc.vector.tensor_tensor(out=lz[:], in0=lz[:],
                                in1=logmv[:, :, 124:128], op=OP.subtract)
        nc.vector.tensor_add(out=lz[:], in0=lz[:], in1=lognv[:, :, 0:4])
        lzs = const.tile([1, 1], f32)
        nc.vector.reduce_sum(out=lzs[:], in_=lz[:], axis=AX.X)
        nps = pstat.tile([1, H * Bl], f32, tag="st0")
        nc.tensor.matmul(out=nps[:, :1], lhsT=ones_colf[:], rhs=numcol[:],
                         start=True, stop=True)
        res = const.tile([1, 1], f32)
        nc.vector.tensor_tensor(out=res[:], in0=nps[:, :1], in1=lzs[:],
                                op=OP.subtract)
        nc.sync.dma_start(out=outv[:, :], in_=res[:])

    nc.compile()
    return nc


def _get_nc():
    global _NC
    if _NC is None:
        _NC = _build()
    return _NC


def make_in_maps(inputs):
    em = np.ascontiguousarray(np.asarray(inputs["emissions"],
                                         dtype=np.float32))
    tags = np.asarray(inputs["tags"]).astype(np.int32)
    st = np.asarray(inputs["start_transitions"], dtype=np.float32)
    en = np.asarray(inputs["end_transitions"], dtype=np.float32)
    tr = np.ascontiguousarray(np.asarray(inputs["transitions"],
                                         dtype=np.float32))
    tssev = np.concatenate(
        [tr.ravel(), st, en, np.zeros(1, np.float32)]).astype(
        np.float32).reshape(TSSE_N, 1)
    s_i = np.arange(S)[:, None]
    b_i = np.arange(Bl)[None, :]
    in_maps = []
    for c in range(NCORES):
        tg = tags[:, c * Bl:(c + 1) * Bl]
        emi = ((s_i * Bl + b_i) * T + tg).astype(np.int32).reshape(128, 128)
        tse = np.full(128 * 129, TSSE_PAD, np.int32)
        tse[:511 * Bl] = (tg[:-1] * T + tg[1:]).astype(np.int32).ravel()
        tse[511 * Bl:511 * Bl + Bl] = T * T + tg[0]
        tse[511 * Bl + Bl:511 * Bl + 2 * Bl] = T * T + T + tg[-1]
        in_maps.append({
            "em": np.ascontiguousarray(em[:, c * Bl:(c + 1) * Bl, :]),
            "transm": tr,
            "startv": st.reshape(T, 1),
            "endv": en.reshape(T, 1),
            "emidx": emi,
            "tssev": tssev,
            "tsseidx": tse.reshape(128, 129),
        })
    return in_maps


def _numpy_fallback(inputs):
    """Exact float64 port of the reference (handles arbitrary masks)."""
    em = np.asarray(inputs["emissions"], dtype=np.float64)
    tags = np.asarray(inputs["tags"]).astype(np.int64)
    mask = np.asarray(inputs["mask"]).astype(bool)
    st = np.asarray(inputs["start_transitions"], dtype=np.float64)
    en = np.asarray(inputs["end_transitions"], dtype=np.float64)
    tr = np.asarray(inputs["transitions"], dtype=np.float64)
    Sl, Bn = tags.shape
    mask_f = mask.astype(np.float64)
    emit = np.take_along_axis(em, tags[:, :, None], axis=2)[:, :, 0]
    trsc = tr[tags[:-1], tags[1:]]
    score = st[tags[0]] + emit[0]
    score = score + ((trsc + emit[1:]) * mask_f[1:]).sum(0)
    seq_ends = mask.astype(np.int64).sum(0) - 1
    score = score + en[tags[seq_ends, np.arange(Bn)]]
    alpha = st[None, :] + em[0]
    for s in range(1, Sl):
        nxt = alpha[:, :, None] + tr[None] + em[s][:, None, :]
        mx = nxt.max(axis=1)
        nxt = mx + np.log(np.exp(nxt - mx[:, None, :]).sum(axis=1))
        alpha = np.where(mask[s][:, None], nxt, alpha)
    z = alpha + en[None, :]
    mz = z.max(axis=1)
    logZ = mz + np.log(np.exp(z - mz[:, None]).sum(axis=1))
    return np.asarray((score - logZ).sum(), dtype=np.float32)


def run_device(inputs, trace=False, trace_kwargs=None):
    from concourse.bass_utils import run_bass_kernel_spmd
    nc = _get_nc()
    in_maps = make_in_maps(inputs)
    br = run_bass_kernel_spmd(nc, in_maps, list(range(NCORES)),
                              trace=trace, **(trace_kwargs or {}))
    total = np.float32(
        sum(float(br.results[i]["out"][0, 0]) for i in range(NCORES)))
    return np.asarray(total, dtype=np.float32), br


def kernel(**inputs):
    mask = np.asarray(inputs["mask"])
    if not bool(mask.all()):
        return _numpy_fallback(inputs)
    val, _ = run_device(inputs, trace=False)
    return val


# revision 15
# speedup vs baseline: 2.1574x; 2.1574x over previous
"""Trainium2 Bass kernel for the BiLSTM-CRF loss (sum reduction).

Strategy (v2):
- Data-parallel: batch 256 sharded as 32 per NeuronCore across 8 cores.
- Normalizer (forward algorithm) in LINEAR space: alpha_{s+1} =
  exp(em_{s+1}) .* (E^T alpha_s), E = exp(transitions). 32 concurrent
  segments of 16 steps (Birkhoff contraction => 6 burn-in rounds on the
  previous segment's tail) -> 22 matmul rounds of width 1024.
- Emission pipeline: slot-major loads (slot j = every 16th timestep) as
  [128,8,128] f32 tiles with 4KB contiguous HBM lines -> scalar Exp
  f32->bf16 in natural layout -> DMA-xbar transpose (bf16) directly into
  erm[t, seg*32+b]. No PE transposes, no DVE casts; the serial chain is
  gated only by the HBM stream.
- Per-segment growth via boundary column sums; fp32 range kept by 2
  delayed column rescales.
- Numerator: two indirect-DMA element gathers + reductions, overlapped.

kernel() contract: full unsharded inputs in, full output (scalar) out.
"""
import numpy as np

S, B, T = 512, 256, 128
NCORES, Bl = 8, 32
NSEG, SEGLEN = 32, 16
BURN = 4
NR = BURN + SEGLEN                   # 20 rounds
NSLOT = SEGLEN
RESC_APPLY = [9, 15]
C_RESC = 2.0 ** -46
RESC_LOGSUM = len(RESC_APPLY) * 46 * float(np.log(2.0))
INIT_BURN = 2.0 ** -30
TSSE_N = T * T + T + T + 1           # 16641: trans | start | end | 0.0
TSSE_PAD = TSSE_N - 1
# slots in need-time order: burn-in tails first, then slot 0, then 1..
LOAD_ORDER = list(range(SEGLEN - (BURN - 1), SEGLEN)) + \
    list(range(SEGLEN - (BURN - 1)))

_NC = None


def _build():
    import concourse.bass as bass
    import concourse.tile as tile
    from concourse import bacc, mybir
    from contextlib import ExitStack

    f32 = mybir.dt.float32
    bf16 = mybir.dt.bfloat16
    i32 = mybir.dt.int32
    AF = mybir.ActivationFunctionType
    OP = mybir.AluOpType
    AX = mybir.AxisListType

    nc = bacc.Bacc("TRN2", target_bir_lowering=False, debug=False,
                   num_devices=NCORES)

    em = nc.dram_tensor("em", [S, Bl, T], f32, kind="ExternalInput")
    transm = nc.dram_tensor("transm", [T, T], f32, kind="ExternalInput")
    startv = nc.dram_tensor("startv", [T, 1], f32, kind="ExternalInput")
    endv = nc.dram_tensor("endv", [T, 1], f32, kind="ExternalInput")
    emidx = nc.dram_tensor("emidx", [128, 128], i32, kind="ExternalInput")
    tssev = nc.dram_tensor("tssev", [TSSE_N, 1], f32, kind="ExternalInput")
    tsseidx = nc.dram_tensor("tsseidx", [128, 129], i32, kind="ExternalInput")
    outv = nc.dram_tensor("out", [1, 1], f32, kind="ExternalOutput")

    H = NSEG // 2

    with tile.TileContext(nc) as tc, ExitStack() as ctx:
        const = ctx.enter_context(tc.tile_pool(name="const", bufs=1))
        stage = ctx.enter_context(tc.tile_pool(name="stage", bufs=4))
        pchain = ctx.enter_context(tc.tile_pool(name="pchain", bufs=2,
                                                space="PSUM"))
        pstat = ctx.enter_context(tc.tile_pool(name="pstat", bufs=2,
                                               space="PSUM"))

        # ---------- constants ----------
        ones_col = const.tile([128, 1], bf16)
        nc.vector.memset(ones_col[:], 1.0)
        ones_colf = const.tile([128, 1], f32)
        nc.vector.memset(ones_colf[:], 1.0)

        tr_sb = const.tile([128, 128], f32)
        nc.sync.dma_start(out=tr_sb[:], in_=transm[:, :])
        E_hi = const.tile([128, 128], bf16)
        nc.scalar.activation(E_hi[:], tr_sb[:], AF.Exp)
        st_sb = const.tile([128, 1], f32)
        nc.sync.dma_start(out=st_sb[:], in_=startv[:, :])
        Estart = const.tile([128, 1], f32)
        nc.scalar.activation(Estart[:], st_sb[:], AF.Exp)
        en_sb = const.tile([128, 1], f32)
        nc.sync.dma_start(out=en_sb[:], in_=endv[:, :])
        Eend = const.tile([128, 1], bf16)
        nc.scalar.activation(Eend[:], en_sb[:], AF.Exp)

        # ---------- numerator: indirect gathers ----------
        emidx_sb = const.tile([128, 128], i32)
        nc.sync.dma_start(out=emidx_sb[:], in_=emidx[:, :])
        tsseidx_sb = const.tile([128, 129], i32)
        nc.sync.dma_start(out=tsseidx_sb[:], in_=tsseidx[:, :])
        gem = const.tile([128, 128], f32)
        nc.gpsimd.indirect_dma_start(
            out=gem[:], out_offset=None,
            in_=bass.AP(tensor=em, offset=0,
                        ap=[[1, S * Bl * T], [1, 1]]),
            in_offset=bass.IndirectOffsetOnAxis(ap=emidx_sb[:], axis=0))
        gts = const.tile([128, 129], f32)
        nc.gpsimd.indirect_dma_start(
            out=gts[:], out_offset=None,
            in_=bass.AP(tensor=tssev, offset=0,
                        ap=[[1, TSSE_N], [1, 1]]),
            in_offset=bass.IndirectOffsetOnAxis(ap=tsseidx_sb[:], axis=0))

        # ---------- chain state + emission storage ----------
        # Band-major free layout: chain column u = 128*bl + 4*g + bh where
        # the batch index b = 8*bh + bl and g is the segment. Segment g
        # occupies p = u%128 in [4g, 4g+4) for each band bl = u//128.
        A2 = const.tile([128, NSEG * Bl], bf16)
        nc.vector.memset(A2[:], INIT_BURN)
        A2v = A2.rearrange("t (g p) -> t g p", g=8)
        erm = const.tile([128, NSLOT, NSEG * Bl], bf16)
        ermv = erm.rearrange("t s (g p) -> t s g p", g=8)

        n_sb = const.tile([1, NSEG * Bl], f32)
        m_sb = const.tile([1, NSEG * Bl], f32)
        fin_sb = const.tile([1, Bl], f32)
        n_sbv = n_sb.rearrange("q (g p) -> q g p", g=8)
        m_sbv = m_sb.rearrange("q (g p) -> q g p", g=8)

        # em[s=16g+j, b=8*bh+bl, t] -> slot j, DRAM iter (g, bh, bl*t);
        # SBUF row r = g*32+b lands at partition r//8, band r%8.
        emr = em[:, :, :].rearrange("(g j) (bh bl) t -> j g bh (bl t)",
                                    j=NSLOT, bh=4)

        def emit_load(j, eng):
            nat = stage.tile([128, 8, 128], f32, tag="nat")
            eng.dma_start(out=nat[:], in_=emr[j])
            return nat

        def emit_expt(j, nat):
            natb = stage.tile([128, 8, 128], bf16, tag="natb")
            nc.scalar.activation(natb[:], nat[:], AF.Exp)
            # one xbar transpose per slot; 3D out scatters each 128-band:
            # erm[t, j, 128*gi + p] = natb[p, gi, t]
            nc.sync.dma_start_transpose(
                out=ermv[:, j, :, :],
                in_=natb.rearrange("p a b -> p (a b)"))

        def emit_round(r):
            if r < BURN - 1:
                slot, sh = SEGLEN - (BURN - 1) + r, -4
            elif r == BURN - 1:
                slot, sh = 0, 0
            else:
                jj = r - BURN
                slot = (jj + 1) % SEGLEN
                sh = 4 if jj == SEGLEN - 1 else 0
            if r < BURN:
                pa, pb = 4, 128      # exclude seg 0
            elif r < NR - 1:
                pa, pb = 0, 128
            else:
                pa, pb = 0, 124      # exclude seg 31
            pw = pb - pa
            for hb, tg in ((0, "psA"), (1, "psB")):
                bs = slice(4 * hb, 4 * hb + 4)
                ps = pchain.tile([128, 4 * 128], f32, tag=tg)
                nc.tensor.matmul(out=ps[:, :4 * pw], lhsT=E_hi[:],
                                 rhs=A2v[:, bs, pa:pb],
                                 start=True, stop=True)
                nc.vector.tensor_tensor(
                    out=A2v[:, bs, pa:pb],
                    in0=ps[:, :4 * pw].rearrange("t (g p) -> t g p", p=pw),
                    in1=ermv[:, slot, bs, pa + sh:pb + sh],
                    op=OP.mult)
            if r in RESC_APPLY:
                nc.vector.tensor_scalar_mul(A2[:], A2[:], C_RESC)
            if r == BURN - 1:
                for h, tg in enumerate(("st0", "st1")):
                    cs = pstat.tile([1, 8 * 64], f32, tag=tg)
                    nc.tensor.matmul(out=cs[:], lhsT=ones_col[:],
                                     rhs=A2v[:, :, 64 * h:64 * (h + 1)],
                                     start=True, stop=True)
                    nc.vector.tensor_copy(
                        out=n_sbv[:, :, 64 * h:64 * (h + 1)],
                        in_=cs.rearrange("q (g p) -> q g p", p=64))
            if r == NR - 2:
                m31 = pstat.tile([1, 8 * 64], f32, tag="st0")
                nc.tensor.matmul(out=m31[:, :32], lhsT=ones_col[:],
                                 rhs=A2v[:, :, 124:128],
                                 start=True, stop=True)
                nc.vector.tensor_copy(
                    out=m_sbv[:, :, 124:128],
                    in_=m31[:, :32].rearrange("q (g p) -> q g p", p=4))
                fin = pstat.tile([1, 8 * 64], f32, tag="st1")
                nc.tensor.matmul(out=fin[:, :32], lhsT=Eend[:],
                                 rhs=A2v[:, :, 124:128],
                                 start=True, stop=True)
                nc.vector.tensor_copy(out=fin_sb[:], in_=fin[:, :32])
            if r == NR - 1:
                mm0 = pstat.tile([1, 8 * 64], f32, tag="st0")
                nc.tensor.matmul(out=mm0[:], lhsT=ones_col[:],
                                 rhs=A2v[:, :, 0:64], start=True, stop=True)
                nc.vector.tensor_copy(
                    out=m_sbv[:, :, 0:64],
                    in_=mm0.rearrange("q (g p) -> q g p", p=64))
                mm1 = pstat.tile([1, 8 * 64], f32, tag="st1")
                nc.tensor.matmul(out=mm1[:, :8 * 60], lhsT=ones_col[:],
                                 rhs=A2v[:, :, 64:124],
                                 start=True, stop=True)
                nc.vector.tensor_copy(
                    out=m_sbv[:, :, 64:124],
                    in_=mm1[:, :8 * 60].rearrange("q (g p) -> q g p", p=60))

        # ---------- emission + chain ----------
        engs = [nc.scalar, nc.gpsimd]
        LA = 3  # load lookahead; must stay < stage bufs to avoid deadlock
        nats = {}
        for i in range(len(LOAD_ORDER) + LA):
            if i < len(LOAD_ORDER):
                j = LOAD_ORDER[i]
                nats[j] = emit_load(j, engs[i % 2])
            if i >= LA:
                jj = LOAD_ORDER[i - LA]
                emit_expt(jj, nats[jj])
        for r in range(BURN - 1):
            emit_round(r)
        # alpha_0 = start .* exp(em_0) replaces seg 0's burn-in state
        nc.vector.tensor_scalar_mul(A2v[:, :, 0:4], ermv[:, 0, :, 0:4],
                                    Estart[:])
        for r in range(BURN - 1, NR):
            emit_round(r)

        # ---------- final assembly ----------
        gsum1 = const.tile([128, 1], f32)
        nc.vector.reduce_sum(out=gsum1[:], in_=gem[:], axis=AX.X)
        gsum2 = const.tile([128, 1], f32)
        nc.vector.reduce_sum(out=gsum2[:], in_=gts[:], axis=AX.X)
        numcol = const.tile([128, 1], f32)
        nc.vector.tensor_add(out=numcol[:], in0=gsum1[:], in1=gsum2[:])
        logn = const.tile([1, NSEG * Bl], f32)
        nc.scalar.activation(logn[:], n_sb[:], AF.Ln)
        logm = const.tile([1, NSEG * Bl], f32)
        nc.scalar.activation(logm[:], m_sb[:], AF.Ln)
        grow = const.tile([1, NSEG * Bl], f32)
        nc.vector.tensor_tensor(out=grow[:], in0=logm[:], in1=logn[:],
                                op=OP.subtract)
        nc.vector.tensor_scalar_add(grow[:], grow[:], RESC_LOGSUM)
        # band-major u = 128*bl + 4*g + bh; reduce over g, out b' = 4*bl+bh
        growb = const.tile([1, Bl], f32)
        nc.vector.reduce_sum(
            out=growb[:],
            in_=grow.rearrange("q (bl g bh) -> q bl bh g", bl=8, bh=4),
            axis=AX.X)
        logfin = const.tile([1, Bl], f32)
        nc.scalar.activation(logfin[:], fin_sb[:], AF.Ln)
        lz = const.tile([1, Bl], f32)
        nc.vector.tensor_add(out=lz[:], in0=growb[:], in1=logfin[:])
        logmv = logm.rearrange("q (g p) -> q g p", g=8)
        lognv = logn.rearrange("q (g p) -> q g p", g=8)
        nc.vector.tensor_tensor(out=lz[:], in0=lz[:],
                                in1=logmv[:, :, 124:128], op=OP.subtract)
        nc.vector.tensor_add(out=lz[:], in0=lz[:], in1=lognv[:, :, 0:4])
        lzs = const.tile([1, 1], f32)
        nc.vector.reduce_sum(out=lzs[:], in_=lz[:], axis=AX.X)
        nps = pstat.tile([1, H * Bl], f32, tag="st0")
        nc.tensor.matmul(out=nps[:, :1], lhsT=ones_colf[:], rhs=numcol[:],
                         start=True, stop=True)
        res = const.tile([1, 1], f32)
        nc.vector.tensor_tensor(out=res[:], in0=nps[:, :1], in1=lzs[:],
                                op=OP.subtract)
        nc.sync.dma_start(out=outv[:, :], in_=res[:])

    nc.compile()
    return nc


def _get_nc():
    global _NC
    if _NC is None:
        _NC = _build()
    return _NC


def make_in_maps(inputs):
    em = np.ascontiguousarray(np.asarray(inputs["emissions"],
                                         dtype=np.float32))
    tags = np.asarray(inputs["tags"]).astype(np.int32)
    st = np.asarray(inputs["start_transitions"], dtype=np.float32)
    en = np.asarray(inputs["end_transitions"], dtype=np.float32)
    tr = np.ascontiguousarray(np.asarray(inputs["transitions"],
                                         dtype=np.float32))
    tssev = np.concatenate(
        [tr.ravel(), st, en, np.zeros(1, np.float32)]).astype(
        np.float32).reshape(TSSE_N, 1)
    s_i = np.arange(S)[:, None]
    b_i = np.arange(Bl)[None, :]
    in_maps = []
    for c in range(NCORES):
        tg = tags[:, c * Bl:(c + 1) * Bl]
        emi = ((s_i * Bl + b_i) * T + tg).astype(np.int32).reshape(128, 128)
        tse = np.full(128 * 129, TSSE_PAD, np.int32)
        tse[:511 * Bl] = (tg[:-1] * T + tg[1:]).astype(np.int32).ravel()
        tse[511 * Bl:511 * Bl + Bl] = T * T + tg[0]
        tse[511 * Bl + Bl:511 * Bl + 2 * Bl] = T * T + T + tg[-1]
        in_maps.append({
            "em": np.ascontiguousarray(em[:, c * Bl:(c + 1) * Bl, :]),
            "transm": tr,
            "startv": st.reshape(T, 1),
            "endv": en.reshape(T, 1),
            "emidx": emi,
            "tssev": tssev,
            "tsseidx": tse.reshape(128, 129),
        })
    return in_maps


def _numpy_fallback(inputs):
    """Exact float64 port of the reference (handles arbitrary masks)."""
    em = np.asarray(inputs["emissions"], dtype=np.float64)
    tags = np.asarray(inputs["tags"]).astype(np.int64)
    mask = np.asarray(inputs["mask"]).astype(bool)
    st = np.asarray(inputs["start_transitions"], dtype=np.float64)
    en = np.asarray(inputs["end_transitions"], dtype=np.float64)
    tr = np.asarray(inputs["transitions"], dtype=np.float64)
    Sl, Bn = tags.shape
    mask_f = mask.astype(np.float64)
    emit = np.take_along_axis(em, tags[:, :, None], axis=2)[:, :, 0]
    trsc = tr[tags[:-1], tags[1:]]
    score = st[tags[0]] + emit[0]
    score = score + ((trsc + emit[1:]) * mask_f[1:]).sum(0)
    seq_ends = mask.astype(np.int64).sum(0) - 1
    score = score + en[tags[seq_ends, np.arange(Bn)]]
    alpha = st[None, :] + em[0]
    for s in range(1, Sl):
        nxt = alpha[:, :, None] + tr[None] + em[s][:, None, :]
        mx = nxt.max(axis=1)
        nxt = mx + np.log(np.exp(nxt - mx[:, None, :]).sum(axis=1))
        alpha = np.where(mask[s][:, None], nxt, alpha)
    z = alpha + en[None, :]
    mz = z.max(axis=1)
    logZ = mz + np.log(np.exp(z - mz[:, None]).sum(axis=1))
    return np.asarray((score - logZ).sum(), dtype=np.float32)


def run_device(inputs, trace=False, trace_kwargs=None):
    from concourse.bass_utils import run_bass_kernel_spmd
    nc = _get_nc()
    in_maps = make_in_maps(inputs)
    br = run_bass_kernel_spmd(nc, in_maps, list(range(NCORES)),
                              trace=trace, **(trace_kwargs or {}))
    total = np.float32(
        sum(float(br.results[i]["out"][0, 0]) for i in range(NCORES)))
    return np.asarray(total, dtype=np.float32), br


def kernel(**inputs):
    mask = np.asarray(inputs["mask"])
    if not bool(mask.all()):
        return _numpy_fallback(inputs)
    val, _ = run_device(inputs, trace=False)
    return val
